# revision 4
# baseline (speedup 1.0000x reference)
"""Trainium2 Bass kernel for ChannelAttention, v2 (single-read + fp16 store).

    k      = einsum('bcit,i->bct', signals, alpha)          # [B, C, T]
    scores = einsum('bct,ts,bds->bcd', k, Wc, k)            # [B, C, C]
    att    = softmax(scores, axis=-1)
    out    = einsum('bci,bint->bcnt', att, signals)         # [B, C, N, T]

Sharding: data-parallel over batch B=16 across 8 cores (2 per core).

Per-core plan (vs the 96 MiB/core double-read baseline):
  - signals read from HBM ONCE (32 MiB fp32), streamed into a per-batch
    fp16 SBUF resident tile R[(h c)=128, (n' t)=32768], n = h*256 + n'.
  - Phase A (k) on the PE from R: per n'-block n0, a small matmul with
    stationary selector sel_n0[p=(h c), c'] = alpha[h*256+n0] * (c==c'),
    accumulating k[c',t] over 256 blocks in PSUM.  sel table (4 MiB fp16)
    is built on-device from alpha via a PSUM broadcast matmul + 256 DVE
    tensor_scalar ops (interleaved with the stream).
  - Phase B: kT via PE transpose, scores = (k Wc) k^T, row softmax,
    att -> fp16, attT packed block-diagonally into [128,128] (both
    n-halves at once).
  - Phase C: out[(h c), :] = attT2.T @ R in 512-col matmuls; PSUM ->
    fp16 staging (ACT/DVE copies) -> HBM as fp16 (host upcasts).
  HBM traffic/core: 32 MiB in + 16 MiB out = 48 MiB (~134 us floor at
  358 GB/s) vs 96 MiB before.

Emission order is chosen for per-engine FIFO cleanliness:
  b0 stream+A | b0 B | b1 stream (DMA+downcast only) | b0 C (ACT copies
  only) | b1 A matmuls | b1 B | b1 C (DVE+ACT copies).
"""

import numpy as np
from contextlib import ExitStack

import concourse.bass as bass
import concourse.bacc as bacc
import concourse.tile as tile
import concourse.mybir as mybir
from concourse.bass_utils import run_bass_kernel_spmd
from concourse.masks import make_identity

B, C, N, T = 16, 64, 512, 128
NCORES = 8
BPC = B // NCORES
P = 128
NH = N // 2                # 256 n'-values per half
NT2 = NH * T               # 32768 cols per resident tile
CK = 2048                  # stream chunk cols (16 n'-blocks)
NCHUNK = NT2 // CK         # 16 chunks per batch
BLK = CK // T              # 16 n'-blocks per chunk
FP32 = mybir.dt.float32
FP16 = mybir.dt.float16

_PROGRAM_CACHE = {}


def _build_program() -> bass.Bass:
    nc = bacc.Bacc(None)
    sig_h = nc.declare_dram_parameter("signals", [BPC, C, N, T], FP32, isOutput=False)
    wc_h = nc.declare_dram_parameter("Wc", [T, T], FP32, isOutput=False)
    al_h = nc.declare_dram_parameter("alpha", [N], FP32, isOutput=False)
    out_h = nc.declare_dram_parameter("out", [BPC, C, N, T], FP16, isOutput=True)

    # per-batch, per-half [c, (n' t)] views; n = h*256 + n'.  SBUF rows
    # (h*64 + c) are fed by two DMAs per chunk, one per half.
    sig_hc = [
        sig_h.ap()[b].rearrange("c (h np) t -> h c (np t)", h=2)
        for b in range(BPC)
    ]
    out_hc = [
        out_h.ap()[b].rearrange("c (h np) t -> h c (np t)", h=2)
        for b in range(BPC)
    ]

    with ExitStack() as ctx:
        tc = ctx.enter_context(tile.TileContext(nc))
        singles = ctx.enter_context(tc.tile_pool(name="singles", bufs=1))
        rpool = ctx.enter_context(tc.tile_pool(name="rpool", bufs=2))
        opool = ctx.enter_context(tc.tile_pool(name="opool", bufs=3))
        small = ctx.enter_context(tc.tile_pool(name="small", bufs=2))
        pa = ctx.enter_context(tc.tile_pool(name="pa", bufs=1, space="PSUM"))
        pk = ctx.enter_context(tc.tile_pool(name="pk", bufs=1, space="PSUM"))
        pb = ctx.enter_context(tc.tile_pool(name="pb", bufs=2, space="PSUM"))
        po = ctx.enter_context(tc.tile_pool(name="po", bufs=4, space="PSUM"))

        # ---- constants
        wc_sb = singles.tile([T, T], FP32)
        nc.sync.dma_start(out=wc_sb, in_=wc_h.ap())
        al_sb = singles.tile([1, N], FP32)
        nc.sync.dma_start(out=al_sb, in_=al_h.ap().rearrange("(o n) -> o n", o=1))

        id64f = singles.tile([64, 64], FP32)
        make_identity(nc, id64f)
        id64h = singles.tile([64, 64], FP16)
        make_identity(nc, id64h)

        # D[p, c'] = (p % 64 == c'), two stacked I64
        d_sb = singles.tile([P, 64], FP16)
        nc.vector.tensor_copy(d_sb[0:64, :], id64h)
        nc.vector.tensor_copy(d_sb[64:128, :], id64h)

        # a2[p, n0] = alpha[(p//64)*256 + n0] via two K=1 broadcast matmuls
        mask0 = singles.tile([1, P], FP32)
        mask1 = singles.tile([1, P], FP32)
        nc.vector.memset(mask0, 0.0)
        nc.vector.memset(mask0[0:1, 0:64], 1.0)
        nc.vector.memset(mask1, 0.0)
        nc.vector.memset(mask1[0:1, 64:128], 1.0)
        a2_ps = pa.tile([P, NH], FP32, tag="pa")
        nc.tensor.matmul(a2_ps, lhsT=mask0, rhs=al_sb[:, 0:NH], start=True, stop=False)
        nc.tensor.matmul(a2_ps, lhsT=mask1, rhs=al_sb[:, NH:N], start=False, stop=True)
        a2_sb = singles.tile([P, NH], FP32)
        nc.vector.tensor_copy(a2_sb, a2_ps)

        # selector table: sel[p, n0*64 + c'] = a2[p, n0] * D[p, c']
        sel_sb = singles.tile([P, NH * 64], FP16)

        resid = []   # per-batch resident tiles

        CKB = 8192               # big-chunk cols (32 KiB/row descriptors)
        NBCH = NT2 // CKB        # 4 chunks per batch

        def stream_batch(b, with_sel, with_mms):
            R = rpool.tile([P, NT2], FP16, tag="R", name=f"R{b}")
            resid.append(R)
            kp = pk.tile([64, T], FP32, tag="k", name=f"k{b}") if with_mms else None
            for j in range(NBCH):
                if with_sel:
                    # sel[p, n0*64+c'] = a2[p, n0] * D[p, c'], quarter at a time
                    nq = NH // NBCH
                    q0 = j * nq
                    sv = sel_sb[:, q0 * 64:(q0 + nq) * 64].rearrange(
                        "p (n c) -> p n c", n=nq
                    )
                    nc.vector.tensor_tensor(
                        sv,
                        a2_sb[:, q0:q0 + nq].unsqueeze(2).broadcast_to([P, nq, 64]),
                        d_sb.unsqueeze(1).broadcast_to([P, nq, 64]),
                        mybir.AluOpType.mult,
                    )
                # casting SWDGE DMA: HBM fp32 -> SBUF fp16, no staging
                nc.gpsimd.dma_start(
                    out=R[0:64, j * CKB:(j + 1) * CKB],
                    in_=sig_hc[b][0, :, j * CKB:(j + 1) * CKB],
                )
                nc.gpsimd.dma_start(
                    out=R[64:128, j * CKB:(j + 1) * CKB],
                    in_=sig_hc[b][1, :, j * CKB:(j + 1) * CKB],
                )
                if with_mms:
                    phase_a_chunk(b, R, kp, j)
            return R, kp

        def phase_a_chunk(b, R, kp, j):
            nblk = CKB // T
            for u in range(nblk):
                n0 = j * nblk + u
                nc.tensor.matmul(
                    kp,
                    lhsT=sel_sb[:, n0 * 64:(n0 + 1) * 64],
                    rhs=R[:, n0 * T:(n0 + 1) * T],
                    start=(n0 == 0),
                    stop=(n0 == NH - 1),
                )

        def phase_b(b, kp):
            k_sb = small.tile([64, T], FP32, tag="ksb", name=f"ksb{b}")
            nc.vector.tensor_copy(k_sb, kp)
            ktp = pb.tile([T, 64], FP32, tag="pb", name=f"ktp{b}")
            nc.tensor.transpose(ktp, k_sb, id64f)
            kt_sb = small.tile([T, 64], FP32, tag="ktsb", name=f"ktsb{b}")
            nc.vector.tensor_copy(kt_sb, ktp)
            kwtp = pb.tile([T, 64], FP32, tag="pb", name=f"kwtp{b}")
            nc.tensor.matmul(kwtp, lhsT=wc_sb, rhs=kt_sb, start=True, stop=True)
            kwt_sb = small.tile([T, 64], FP32, tag="kwtsb", name=f"kwtsb{b}")
            nc.vector.tensor_copy(kwt_sb, kwtp)
            scp = pb.tile([64, 64], FP32, tag="pb", name=f"scp{b}")
            nc.tensor.matmul(scp, lhsT=kwt_sb, rhs=kt_sb, start=True, stop=True)

            mx = small.tile([64, 1], FP32, tag="mx", name=f"mx{b}")
            nmx = small.tile([64, 1], FP32, tag="nmx", name=f"nmx{b}")
            ssum = small.tile([64, 1], FP32, tag="ssum", name=f"ssum{b}")
            rsum = small.tile([64, 1], FP32, tag="rsum", name=f"rsum{b}")
            att_f = small.tile([64, 64], FP32, tag="attf", name=f"attf{b}")
            att_h_f32 = small.tile([64, 64], FP32, tag="atth", name=f"atth{b}")
            nc.vector.reduce_max(out=mx, in_=scp, axis=mybir.AxisListType.X)
            nc.vector.tensor_scalar_mul(nmx, mx, -1.0)
            nc.scalar.activation(
                att_f, scp, mybir.ActivationFunctionType.Exp,
                bias=nmx, scale=1.0, accum_out=ssum,
            )
            nc.vector.reciprocal(rsum, ssum)
            nc.scalar.mul(att_h_f32, att_f, rsum)
            atp = pb.tile([64, 64], FP32, tag="pb", name=f"atp{b}")
            nc.tensor.transpose(atp, att_h_f32, id64f)
            at2 = small.tile([P, P], FP16, tag="at2", name=f"at2{b}")
            nc.vector.memset(at2, 0.0)
            nc.vector.tensor_copy(at2[0:64, 0:64], atp)
            nc.vector.tensor_copy(at2[64:128, 64:128], atp)
            return at2

        GRP = 4096           # out staging cols (fp16)
        QPG = GRP // 512     # 512-col matmuls per group

        def phase_c_group(b, R, at2, g, copy_engines, h1_queue):
            stg = opool.tile([P, GRP], FP16, tag="o", name=f"o{b}_{g}")
            for q in range(QPG):
                ps = po.tile([P, 512], FP32, tag="po", name=f"po{b}_{g}_{q}")
                nc.tensor.matmul(
                    ps, lhsT=at2,
                    rhs=R[:, (g * QPG + q) * 512:(g * QPG + q + 1) * 512],
                    start=True, stop=True,
                )
                eng = copy_engines[q % len(copy_engines)]
                if eng == "v":
                    nc.vector.tensor_copy(stg[:, q * 512:(q + 1) * 512], ps)
                else:
                    nc.scalar.copy(stg[:, q * 512:(q + 1) * 512], ps)
            nc.sync.dma_start(
                out=out_hc[b][0, :, g * GRP:(g + 1) * GRP], in_=stg[0:64, :]
            )
            h1_queue.dma_start(
                out=out_hc[b][1, :, g * GRP:(g + 1) * GRP], in_=stg[64:128, :]
            )

        def phase_c(b, R, at2, copy_engines, h1_queue):
            for g in range(NT2 // GRP):
                phase_c_group(b, R, at2, g, copy_engines, h1_queue)

        # ---- emission schedule.  b1's stream chunks, b0's phase-C groups
        # and b1's phase-A matmuls are interleaved so the PE/DVE/ACT FIFOs
        # never serialize the b1 tail behind the whole of b0's phase C.
        R0, kp0 = stream_batch(0, with_sel=True, with_mms=True)
        at2_0 = phase_b(0, kp0)
        R1 = rpool.tile([P, NT2], FP16, tag="R", name="R1")
        kp1 = pk.tile([64, T], FP32, tag="k", name="k1")
        GPB = (NT2 // GRP) // NBCH      # b0 phase-C groups per b1 chunk
        for j in range(NBCH):
            nc.gpsimd.dma_start(
                out=R1[0:64, j * CKB:(j + 1) * CKB],
                in_=sig_hc[1][0, :, j * CKB:(j + 1) * CKB],
            )
            nc.gpsimd.dma_start(
                out=R1[64:128, j * CKB:(j + 1) * CKB],
                in_=sig_hc[1][1, :, j * CKB:(j + 1) * CKB],
            )
            for g in range(j * GPB, (j + 1) * GPB):
                phase_c_group(0, R0, at2_0, g,
                              copy_engines=["v", "v", "v", "s"],
                              h1_queue=nc.scalar)
            # phase-A matmuls for chunk j at the END of the iteration: by
            # the time the PE drains the C groups ahead, chunk j's DMA has
            # landed, so these neither stall nor head-block anything.
            phase_a_chunk(1, R1, kp1, j)
        at2_1 = phase_b(1, kp1)
        phase_c(1, R1, at2_1, copy_engines=["v", "s"], h1_queue=nc.gpsimd)

    nc.compile()
    return nc


def _get_program() -> bass.Bass:
    if "nc" not in _PROGRAM_CACHE:
        _PROGRAM_CACHE["nc"] = _build_program()
    return _PROGRAM_CACHE["nc"]


def kernel(signals, Wc, alpha, **run_kwargs):
    signals = np.ascontiguousarray(np.asarray(signals, dtype=np.float32))
    Wc = np.ascontiguousarray(np.asarray(Wc, dtype=np.float32))
    alpha = np.ascontiguousarray(np.asarray(alpha, dtype=np.float32))
    assert signals.shape == (B, C, N, T)

    nc = _get_program()
    core_ids = list(range(NCORES))
    in_maps = [
        {
            "signals": signals[j * BPC:(j + 1) * BPC],
            "Wc": Wc,
            "alpha": alpha,
        }
        for j in range(NCORES)
    ]
    res = run_bass_kernel_spmd(nc, in_maps, core_ids, **run_kwargs)
    out = np.empty((B, C, N, T), dtype=np.float32)
    for j in range(NCORES):
        out[j * BPC:(j + 1) * BPC] = np.asarray(res.results[j]["out"], dtype=np.float32)
    if run_kwargs:
        kernel.last_results = res
    return out
